# revision 1
# baseline (speedup 1.0000x reference)
"""Circular correlation 1D (FFT reference) as a direct 9-tap conv on TRN2.

Math: the reference's ortho-normalized FFT round trip reduces exactly to
    y[b, o, m] = sum_i sum_t K[o, i, t] * x[b, i, (m + t) mod N] + bias[o]
so we compute it as 9 PSUM-accumulated float32r matmuls per 512-col chunk:
    lhsT = K[:, :, t]^T  (shape [c_in=128, c_out=128], c_in on partitions)
    rhs  = x[b, :, m0+t : m0+t+512]    (c_in on partitions)
float32r runs the PE at full rate (1 cycle/row) for free dims >= 256, vs 4
cycles/row for plain fp32 — per-core PE floor is ~61.4 us for this shape.

Sharding: pure data-parallel over batch — 32 batches / 8 cores = 4 each.
Each core computes its full [c_out=128, N=4096] slab; no collectives.

Pipeline structure (tuned against the instruction cost-model timeline; the
PE runs gap-free from the first to the last matmul):
- head: DMAs are emitted in critical-path order — w taps 0-2, then x batch 0
  cols 0-519 (exactly chunk 0's reads), then w taps 3-8, then the rest of
  x batch 0 in 512/512/remainder pieces. First matmul issues ~4.3 us in,
  with tap t and chunk c data always landing just ahead of use.
- x arrives with an 8-col circular halo (host-padded) so every matmul rhs is
  a contiguous SBUF slice; batches 1-3 stream in halves during compute.
- outputs stage in quarter-batch SBUF tiles (never-reused slots); the final
  quarter is written out as 512+256+256 cols so the last DMA is short.
- dummy bf16 LDWEIGHTS absorb DMA-completion waits onto the PE; remaining
  multi-wait instructions are legalized by Bacc.compile()'s
  generate_event_semaphores pass (TRN2 allows one sync wait per engine
  instruction — building with plain bass.Bass() fails walrus codegen).
"""

import sys

if "/opt/trn_rl_repo" not in sys.path:
    sys.path.insert(0, "/opt/trn_rl_repo")

import numpy as np

import concourse.bass as bass
import concourse.mybir as mybir
import concourse.tile as tile
from concourse import bacc
from concourse.bass_utils import run_bass_kernel_spmd

B, C_IN, C_OUT, KS, N = 32, 128, 128, 9, 4096
N_CORES = 8
BPC = B // N_CORES  # batches per core
CHUNK = 512  # one PSUM bank of fp32; max fp32 moving free-dim
HALO = KS - 1
OUT_PARTS = 4  # quarter-batch output staging
W_SPLIT = (3, 6)  # w DMA pieces (taps): first piece unblocks chunk 0
X0_PIECES = (520, 512, 512)  # leading x-batch-0 pieces (remainder appended)
TAIL_SUBS = (512, 256, 256)  # final quarter written in shrinking pieces
# the tail pieces must exactly tile the final part, or chunks are silently
# skipped; every piece must be >=256 cols (fp32r drops to 1/4 rate below)
assert sum(TAIL_SUBS) == N // OUT_PARTS and min(TAIL_SUBS) >= 256

_DT_F32 = mybir.dt.float32
_DT_F32R = mybir.dt.float32r  # full-rate fp32 matmul mode (free dim >= 256)


def build_nc() -> bass.Bass:
    nc = bacc.Bacc()
    x_ext = nc.dram_tensor("x", [BPC, C_IN, N + HALO], _DT_F32R, kind="ExternalInput")
    w_ext = nc.dram_tensor("w", [C_IN, KS * C_OUT], _DT_F32R, kind="ExternalInput")
    b_ext = nc.dram_tensor("b", [C_OUT, 1], _DT_F32, kind="ExternalInput")
    y_ext = nc.dram_tensor("y", [BPC, C_OUT, N], _DT_F32, kind="ExternalOutput")

    with tile.TileContext(nc) as tc:
        with (
            tc.tile_pool(name="const", bufs=1) as cpool,
            # distinct tag per batch: x slots never reused -> no WAR waits
            tc.tile_pool(name="xin", bufs=1) as xpool,
            tc.tile_pool(name="psum", bufs=8, space="PSUM") as ppool,
            # never-reused staging slots: ACT writes carry no WAR waits
            tc.tile_pool(name="out", bufs=OUT_PARTS * BPC - 1) as opool,
            tc.tile_pool(name="tail", bufs=1) as tpool,
        ):
            w_t = cpool.tile([C_IN, KS * C_OUT], _DT_F32R)
            bias_t = cpool.tile([C_OUT, 1], _DT_F32)
            x_tiles = []
            for b in range(BPC):
                xt = xpool.tile([C_IN, N + HALO], _DT_F32R, tag=f"x{b}")
                x_tiles.append(xt)
            wbf = w_t[:].bitcast(mybir.dt.bfloat16)

            def w_piece(t0, npiece):
                sl = slice(t0 * C_OUT, (t0 + npiece) * C_OUT)
                nc.sync.dma_start(out=w_t[:, sl], in_=w_ext[:, sl])
                # dummy bf16 LDWEIGHTS inside this piece: absorbs the DMA
                # wait on the PE queue (fp32r matmuls self-load weights, so
                # the loaded garbage is never used)
                nc.tensor.ldweights(wbf[:, 2 * t0 * C_OUT : 2 * t0 * C_OUT + C_OUT])

            def x_piece(b, s, e):
                nc.sync.dma_start(out=x_tiles[b][:, s:e], in_=x_ext[b, :, s:e])
                xbf = x_tiles[b][:].bitcast(mybir.dt.bfloat16)
                nc.tensor.ldweights(xbf[:, 2 * s : 2 * s + C_OUT])

            # critical-path-ordered head: w piece 1, x0 piece 1 (chunk 0's
            # data), w piece 2, bias, then the rest of x batch 0
            t0 = 0
            w_piece(t0, W_SPLIT[0])
            t0 += W_SPLIT[0]
            cuts0 = [0]
            for p in X0_PIECES:
                cuts0.append(cuts0[-1] + p)
            cuts0.append(N + HALO)
            x_piece(0, cuts0[0], cuts0[1])
            for npiece in W_SPLIT[1:]:
                w_piece(t0, npiece)
                t0 += npiece
            nc.sync.dma_start(out=bias_t[:], in_=b_ext[:])
            bias_warm = cpool.tile([C_OUT, 1], _DT_F32)
            nc.scalar.activation(
                bias_warm[:], bias_t[:], mybir.ActivationFunctionType.Identity
            )
            bias_warm2 = cpool.tile([C_OUT, 1], _DT_F32)
            nc.vector.tensor_scalar_add(bias_warm2[:], bias_t[:], 0.0)
            for s, e in zip(cuts0[1:-1], cuts0[2:]):
                if e > s:
                    x_piece(0, s, e)
            for b in range(1, BPC):
                half = (N + HALO + 1) // 2
                for s, e in ((0, half), (half, N + HALO)):
                    x_piece(b, s, e)

            part = N // OUT_PARTS
            for b in range(BPC):
                x_t = x_tiles[b]
                for h in range(OUT_PARTS):
                    last_part = b == BPC - 1 and h == OUT_PARTS - 1
                    subs = list(TAIL_SUBS) if last_part else [part]
                    off = 0
                    for ui, sub in enumerate(subs):
                        pool_ = tpool if last_part else opool
                        stage = pool_.tile(
                            [C_OUT, sub],
                            _DT_F32,
                            tag=f"tail{ui}" if last_part else "stage",
                        )
                        for cc in range(max(1, sub // CHUNK)):
                            w_cols = min(sub, CHUNK)
                            m0 = h * part + off + cc * w_cols
                            ps = ppool.tile([C_OUT, w_cols], _DT_F32, tag="ps")
                            for t in range(KS):
                                nc.tensor.matmul(
                                    ps[:],
                                    w_t[:, t * C_OUT : (t + 1) * C_OUT],
                                    x_t[:, m0 + t : m0 + t + w_cols],
                                    start=(t == 0),
                                    stop=(t == KS - 1),
                                )
                            osl = stage[:, cc * w_cols : (cc + 1) * w_cols]
                            if last_part and ui >= 1:
                                # final two pieces drain on the (idle) DVE so
                                # the tail chain starts the moment the last
                                # matmul stops; fp32 add is bit-identical to
                                # the ACT bias path
                                nc.vector.tensor_scalar_add(osl, ps[:], bias_t[:])
                            else:
                                nc.scalar.activation(
                                    osl,
                                    ps[:],
                                    mybir.ActivationFunctionType.Identity,
                                    bias=bias_t[:],
                                )
                        nc.sync.dma_start(
                            out=y_ext[b, :, h * part + off : h * part + off + sub],
                            in_=stage[:],
                        )
                        off += sub
    # Legalize: splits any instruction with >1 sync wait into EventSemaphore
    # chains (TRN2 allows one wait per instruction), register alloc, DCE.
    nc.compile()
    return nc


def _prep_inputs(x: np.ndarray, k: np.ndarray, bias: np.ndarray):
    # circular halo so every rhs slice is contiguous in SBUF
    x_pad = np.concatenate([x, x[:, :, :HALO]], axis=-1)
    # w[i, t*C_OUT + o] = k[o, i, t]  -> lhsT slice [:, t*128:(t+1)*128] is [i, o]
    w = np.ascontiguousarray(k.transpose(1, 2, 0)).reshape(C_IN, KS * C_OUT)
    b2 = np.ascontiguousarray(bias.reshape(C_OUT, 1))
    in_maps = [
        {
            "x": np.ascontiguousarray(x_pad[c * BPC : (c + 1) * BPC]),
            "w": w,
            "b": b2,
        }
        for c in range(N_CORES)
    ]
    return in_maps


_NC_CACHE = []


def kernel(**inputs: np.ndarray) -> np.ndarray:
    x = np.asarray(inputs["x"], dtype=np.float32)
    k = np.asarray(inputs["kernel"], dtype=np.float32)
    bias = np.asarray(inputs["bias"], dtype=np.float32)
    assert x.shape == (B, C_IN, N) and k.shape == (C_OUT, C_IN, KS)

    if not _NC_CACHE:
        _NC_CACHE.append(build_nc())
    nc = _NC_CACHE[0]

    in_maps = _prep_inputs(x, k, bias)
    res = run_bass_kernel_spmd(nc, in_maps, list(range(N_CORES)))
    y = np.concatenate([res.results[c]["y"] for c in range(N_CORES)], axis=0)
    return y.astype(np.float32, copy=False)



# revision 7
# speedup vs baseline: 1.2551x; 1.2551x over previous
"""Circular correlation 1D (FFT reference) as a direct 9-tap conv on TRN2,
computed with fp8(e4m3) DoubleRow matmuls.

Math: the reference's ortho-normalized FFT round trip reduces exactly to
    y[b, o, m] = sum_i sum_t K[o, i, t] * x[b, i, (m + t) mod N] + bias[o]

Precision scheme: split x = x_hi + x_lo and K = k_hi + k_lo (each half an
e4m3 rounding + e4m3-rounded residual), and accumulate in fp32 PSUM
    y ~= k_hi*x_hi + k_lo*x_hi + k_hi*x_lo      (the k_lo*x_lo term ~ 1e-3)
Each 128-deep (c_in) contraction tile runs in DoubleRow perf mode, which
contracts TWO k-tiles per pass at 0.5 cycles per output column -- 4x the
fp32r MAC rate.  27 of the 28 half-slots in 14 DR matmuls would be useful;
we drop one correction pair (both k_lo[4]*x_hi and k_hi[4]*x_lo) to land on
13 DR matmuls per 512-col chunk (measured rel err ~1.3e-2 vs the 2e-2 gate;
set DROP_CORR = () for the 14-matmul variant at ~1.3e-3).

DoubleRow operand layout (hardware requires the pair halves at one fixed
stride -- dim1 of a [128, 2, cols] AP; overlapping stride-1 dim1 crashes the
exec unit, so all pairs read adjacent SBUF planes at the SAME column):
    P0[c] = x_hi[c-1],  P1[c] = x_hi[c],  P2[c] = x_lo[c]   (circular in c)
    hh pair (t, t+1):  w=(k_hi[t], k_hi[t+1]),  rhs = planes[0:2] @ col m+t+1
    corr pair t:       w=(k_lo[t], k_hi[t]),    rhs = planes[1:3] @ col m+t
    leftover:          w=(k_hi[8], k_lo[8]),    rhs = planes[1:3] @ col m+8

Sharding: pure data-parallel over batch -- 32 batches / 8 cores = 4 each.
Each core computes its full [c_out=128, N=4096] slab; no collectives.

Pipeline: x planes are host-prepped fp8 (2.3x less input DMA than fp32).
Input DMAs ride the SP (sync) queue, weight/bias/output DMAs the ACT
(scalar) queue so input and output transfers overlap.  A memset-fed run of
dummy fp8 matmuls burns the PE p-state ramp (0.65/1.2 GHz until 3us of
continuous busy) under the DMA head so real matmuls open at 2.4 GHz.
Dummy bf16 LDWEIGHTS after each DMA absorb completion waits on the PE
queue (TRN2 allows one sync wait per engine instruction; Bacc.compile()'s
event-semaphore pass legalizes the rest)."""

import sys

if "/opt/trn_rl_repo" not in sys.path:
    sys.path.insert(0, "/opt/trn_rl_repo")

import ml_dtypes
import numpy as np

import concourse.bass as bass
import concourse.mybir as mybir
import concourse.tile as tile
from concourse import bacc
from concourse.bass_utils import run_bass_kernel_spmd

B, C_IN, C_OUT, KS, N = 32, 128, 128, 9, 4096
N_CORES = 8
BPC = B // N_CORES  # batches per core
CHUNK = 512  # one PSUM bank of fp32
HALO = KS - 1
LEAD = 1  # one leading circular column so P0[c] = x_hi[c-1] stays in range
PITCH = 4108  # plane pitch (LEAD + N + HALO = 4105, padded to a multiple of 4)
COLS_USED = LEAD + N + HALO
OUT_PARTS = 4  # quarter-batch output staging
TAIL_SUBS = (512, 256, 256)  # final quarter written in shrinking pieces
DROP_CORR = (4,)  # correction pairs dropped to reach 13 DR matmuls/chunk
N_WARM = 10  # p-state warmup matmuls (512 cols, plain fp8)
X0_PIECES = (524, 1792, 1792)  # leading x-batch-0 col pieces (remainder appended)

_DT_F8 = mybir.dt.float8e4
_DT_F32 = mybir.dt.float32
_NP_F8 = ml_dtypes.float8_e4m3
_DR = mybir.MatmulPerfMode.DoubleRow


def _pair_table():
    """(plane_lo, col_off) per DR matmul + matching weight-tile order."""
    pairs = []  # (wa_kind, wa_tap, wb_kind, wb_tap, plane_lo, col_off)
    for t in (0, 2, 4, 6):
        pairs.append(("hi", t, "hi", t + 1, 0, t + 1))
    for t in range(KS):
        if t in DROP_CORR:
            continue
        pairs.append(("lo", t, "hi", t, 1, t))
    pairs.append(("hi", 8, "lo", 8, 1, 8))
    return pairs


PAIRS = _pair_table()
NPAIRS = len(PAIRS)


def build_nc() -> bass.Bass:
    nc = bacc.Bacc()
    x_ext = nc.dram_tensor("x", [BPC, C_IN, 3 * PITCH], _DT_F8, kind="ExternalInput")
    w_ext = nc.dram_tensor("w", [C_IN, NPAIRS * 2 * C_OUT], _DT_F8, kind="ExternalInput")
    b_ext = nc.dram_tensor("b", [C_OUT, 1], _DT_F32, kind="ExternalInput")
    y_ext = nc.dram_tensor("y", [BPC, C_OUT, N], _DT_F32, kind="ExternalOutput")

    with tile.TileContext(nc) as tc:
        with (
            tc.tile_pool(name="const", bufs=1) as cpool,
            tc.tile_pool(name="xin", bufs=1) as xpool,
            tc.tile_pool(name="psum", bufs=7, space="PSUM") as ppool,
            tc.tile_pool(name="warm", bufs=1, space="PSUM") as wppool,
            # never-reused staging slots: ACT writes carry no WAR waits
            tc.tile_pool(name="out", bufs=OUT_PARTS * BPC - 1) as opool,
            tc.tile_pool(name="tail", bufs=1) as tpool,
        ):
            w_t = cpool.tile([C_IN, NPAIRS * 2, C_OUT], _DT_F8)
            bias_t = cpool.tile([C_OUT, 1], _DT_F32)
            warm_t = cpool.tile([C_IN, 640], _DT_F8)
            x_tiles = []
            for b in range(BPC):
                xt = xpool.tile([C_IN, 3, PITCH], _DT_F8, tag=f"x{b}")
                x_tiles.append(xt)

            # ---- p-state warmup: memset-fed dummy matmuls keep the PE busy
            # (and ramping) while the head DMAs land
            nc.vector.memset(warm_t[:], 0.0)
            for i in range(N_WARM):
                wps = wppool.tile([C_OUT, CHUNK], _DT_F32, tag="warm")
                nc.tensor.matmul(
                    wps[:], warm_t[:, 512:640], warm_t[:, 0:512], start=True, stop=True
                )

            # ---- head DMAs. Input x on the SP queue; w/bias on the ACT queue
            # so they transfer in parallel.
            def x_piece(b, s, e):
                nc.sync.dma_start(
                    out=x_tiles[b][:, :, s:e],
                    in_=x_ext[b].rearrange("p (three f) -> p three f", three=3)[
                        :, :, s:e
                    ],
                )
                xbf = x_tiles[b][:].bitcast(mybir.dt.bfloat16)
                # dummy bf16 LDWEIGHTS inside the piece's range: absorbs the
                # DMA wait on the PE queue (cost-free; weights never used)
                nc.tensor.ldweights(xbf[:, 0:1, s // 2 : s // 2 + C_OUT])

            nc.scalar.dma_start(out=w_t[:], in_=w_ext[:].rearrange(
                "p (n f) -> p n f", n=NPAIRS * 2))
            wbf = w_t[:].bitcast(mybir.dt.bfloat16)
            nc.tensor.ldweights(wbf[:, 0:2, 0 : C_OUT // 2])
            nc.scalar.dma_start(out=bias_t[:], in_=b_ext[:])

            cuts0 = [0]
            for p in X0_PIECES:
                cuts0.append(cuts0[-1] + p)
            cuts0.append(PITCH)
            for s, e in zip(cuts0[:-1], cuts0[1:]):
                if e > s:
                    x_piece(0, s, e)
            for b in range(1, BPC):
                half = PITCH // 2
                for s, e in ((0, half), (half, PITCH)):
                    x_piece(b, s, e)

            # ---- main loop: per 512-col chunk, 13 PSUM-accumulated DR
            # matmuls, then ACT identity+bias into an SBUF staging slot
            part = N // OUT_PARTS
            for b in range(BPC):
                x_t = x_tiles[b]
                for h in range(OUT_PARTS):
                    last_part = b == BPC - 1 and h == OUT_PARTS - 1
                    subs = list(TAIL_SUBS) if last_part else [part]
                    off = 0
                    for ui, sub in enumerate(subs):
                        pool_ = tpool if last_part else opool
                        stage = pool_.tile(
                            [C_OUT, sub],
                            _DT_F32,
                            tag=f"tail{ui}" if last_part else "stage",
                        )
                        for cc in range(max(1, sub // CHUNK)):
                            w_cols = min(sub, CHUNK)
                            m0 = h * part + off + cc * w_cols
                            ps = ppool.tile([C_OUT, w_cols], _DT_F32, tag="ps")
                            for pi, (_, _, _, _, plo, coff) in enumerate(PAIRS):
                                nc.tensor.matmul(
                                    ps[:],
                                    w_t[:, 2 * pi : 2 * pi + 2, :],
                                    x_t[:, plo : plo + 2, m0 + coff : m0 + coff + w_cols],
                                    start=(pi == 0),
                                    stop=(pi == NPAIRS - 1),
                                    perf_mode=_DR,
                                )
                            osl = stage[:, cc * w_cols : (cc + 1) * w_cols]
                            if last_part and ui >= 1:
                                # final two pieces drain on the (idle) DVE so
                                # the tail starts the moment the PE stops
                                nc.vector.tensor_scalar_add(osl, ps[:], bias_t[:])
                            else:
                                nc.scalar.activation(
                                    osl,
                                    ps[:],
                                    mybir.ActivationFunctionType.Identity,
                                    bias=bias_t[:],
                                )
                        nc.scalar.dma_start(
                            out=y_ext[b, :, h * part + off : h * part + off + sub],
                            in_=stage[:],
                        )
                        off += sub
    # Legalize: splits any instruction with >1 sync wait into EventSemaphore
    # chains (TRN2 allows one wait per instruction), register alloc, DCE.
    nc.compile()
    return nc


def _prep_inputs(x: np.ndarray, k: np.ndarray, bias: np.ndarray):
    x_hi8 = x.astype(_NP_F8)
    x_hi = x_hi8.astype(np.float32)
    x_lo8 = (x - x_hi).astype(_NP_F8)

    idx = np.arange(COLS_USED)
    planes = np.zeros((B, C_IN, 3, PITCH), dtype=_NP_F8)
    planes[:, :, 0, :COLS_USED] = x_hi8[:, :, (idx - 1) % N]
    planes[:, :, 1, :COLS_USED] = x_hi8[:, :, idx % N]
    planes[:, :, 2, :COLS_USED] = x_lo8[:, :, idx % N]
    planes = planes.reshape(B, C_IN, 3 * PITCH)

    k_hi8 = k.astype(_NP_F8)
    k_hi = k_hi8.astype(np.float32)
    k_lo8 = (k - k_hi).astype(_NP_F8)
    ksrc = {"hi": k_hi8, "lo": k_lo8}
    w = np.zeros((C_IN, NPAIRS * 2, C_OUT), dtype=_NP_F8)
    for pi, (ka, ta, kb, tb, _, _) in enumerate(PAIRS):
        w[:, 2 * pi, :] = ksrc[ka][:, :, ta].T  # [i, o]
        w[:, 2 * pi + 1, :] = ksrc[kb][:, :, tb].T
    w = w.reshape(C_IN, NPAIRS * 2 * C_OUT)

    b2 = np.ascontiguousarray(bias.reshape(C_OUT, 1)).astype(np.float32)
    return [
        {
            "x": np.ascontiguousarray(planes[c * BPC : (c + 1) * BPC]),
            "w": w,
            "b": b2,
        }
        for c in range(N_CORES)
    ]


_NC_CACHE = []


def kernel(**inputs: np.ndarray) -> np.ndarray:
    x = np.asarray(inputs["x"], dtype=np.float32)
    k = np.asarray(inputs["kernel"], dtype=np.float32)
    bias = np.asarray(inputs["bias"], dtype=np.float32)
    assert x.shape == (B, C_IN, N) and k.shape == (C_OUT, C_IN, KS)

    if not _NC_CACHE:
        _NC_CACHE.append(build_nc())
    nc = _NC_CACHE[0]

    in_maps = _prep_inputs(x, k, bias)
    res = run_bass_kernel_spmd(nc, in_maps, list(range(N_CORES)))
    y = np.concatenate([res.results[c]["y"] for c in range(N_CORES)], axis=0)
    return y.astype(np.float32, copy=False)


# revision 30
# speedup vs baseline: 1.2978x; 1.0340x over previous
"""Circular correlation 1D (FFT reference) as a direct 9-tap conv on TRN2,
computed with fp8(e4m3) DoubleRow matmuls.

Math: the reference's ortho-normalized FFT round trip reduces exactly to
    y[b, o, m] = sum_i sum_t K[o, i, t] * x[b, i, (m + t) mod N] + bias[o]

Precision scheme: split x = x_hi + x_lo and K = k_hi + k_lo (each half an
e4m3 rounding + e4m3-rounded residual), and accumulate in fp32 PSUM
    y ~= k_hi*x_hi + k_lo*x_hi + k_hi*x_lo      (the k_lo*x_lo term ~ 1e-3)
Each 128-deep (c_in) contraction tile runs in DoubleRow perf mode, which
contracts TWO k-tiles per pass at 0.5 cycles per output column -- 4x the
fp32r MAC rate.  27 of the 28 half-slots in 14 DR matmuls would be useful;
we drop one correction pair (both k_lo[4]*x_hi and k_hi[4]*x_lo) to land on
13 DR matmuls per 512-col chunk (measured rel err ~1.3e-2 vs the 2e-2 gate;
set DROP_CORR = () for the 14-matmul variant at ~1.3e-3).

DoubleRow operand layout (hardware requires the pair halves at one fixed
stride -- dim1 of a [128, 2, cols] AP; overlapping stride-1 dim1 crashes the
exec unit, so all pairs read adjacent SBUF planes at the SAME column):
    P0[c] = x_hi[c-1],  P1[c] = x_hi[c],  P2[c] = x_lo[c]   (circular in c)
    hh pair (t, t+1):  w=(k_hi[t], k_hi[t+1]),  rhs = planes[0:2] @ col m+t+1
    corr pair t:       w=(k_lo[t], k_hi[t]),    rhs = planes[1:3] @ col m+t
    leftover:          w=(k_hi[8], k_lo[8]),    rhs = planes[1:3] @ col m+8

Sharding: pure data-parallel over batch -- 32 batches / 8 cores = 4 each.
Each core computes its full [c_out=128, N=4096] slab; no collectives.

Pipeline: x planes are host-prepped fp8 (2.3x less input DMA than fp32).
Input DMAs ride the SP (sync) queue, weight/bias/output DMAs the ACT
(scalar) queue so input and output transfers overlap.  A memset-fed run of
dummy fp8 matmuls burns the PE p-state ramp (0.65/1.2 GHz until 3us of
continuous busy) under the DMA head so real matmuls open at 2.4 GHz.
Dummy bf16 LDWEIGHTS after each DMA absorb completion waits on the PE
queue (TRN2 allows one sync wait per engine instruction; Bacc.compile()'s
event-semaphore pass legalizes the rest)."""

import sys

if "/opt/trn_rl_repo" not in sys.path:
    sys.path.insert(0, "/opt/trn_rl_repo")

import ml_dtypes
import numpy as np

import concourse.bass as bass
import concourse.mybir as mybir
import concourse.tile as tile
from concourse import bacc
from concourse.bass_utils import run_bass_kernel_spmd

B, C_IN, C_OUT, KS, N = 32, 128, 128, 9, 4096
N_CORES = 8
BPC = B // N_CORES  # batches per core
CHUNK = 512  # one PSUM bank of fp32
HALO = KS - 1
LEAD = 1  # one leading circular column so P0[c] = x_hi[c-1] stays in range
PITCH = 4108  # plane pitch (LEAD + N + HALO = 4105, padded to a multiple of 4)
COLS_USED = LEAD + N + HALO
OUT_PARTS = 4  # quarter-batch output staging
TAIL_SUBS = (512, 256, 256)  # final quarter written in shrinking pieces
DROP_CORR = (4,)  # correction pairs dropped to reach 13 DR matmuls/chunk
WARM_BIG = 13  # 256-col p-state warmup matmuls (plain fp8 over memset junk)
WARM_SMALL = 2  # 128-col trailing warmups for a fine-grained handoff
HEAD_SUBS = (256, 256, 512)  # batch-0 part-0 piece sizes: earliest possible start
X0_PIECES = (264, 260, 512, 1024, 2048)  # x batch-0 col pieces (JIT ladder)
X0_LD = {0: 0, 256: 1, 512: 2, 1024: 3, 2048: 4}  # m0 -> piece first consumed
XH_SPLIT = 2056  # batches 1-3 half split (chunk m0=1536 reads cols <= 2055)
W_SP = 6  # pairs [0, W_SP) ride the SP queue ahead of x0; rest + bias on ACT

_DT_F8 = mybir.dt.float8e4
_DT_F32 = mybir.dt.float32
_NP_F8 = ml_dtypes.float8_e4m3
_DR = mybir.MatmulPerfMode.DoubleRow


def _pair_table():
    """(plane_lo, col_off) per DR matmul + matching weight-tile order."""
    pairs = []  # (wa_kind, wa_tap, wb_kind, wb_tap, plane_lo, col_off)
    for t in (0, 2, 4, 6):
        pairs.append(("hi", t, "hi", t + 1, 0, t + 1))
    for t in range(KS):
        if t in DROP_CORR:
            continue
        pairs.append(("lo", t, "hi", t, 1, t))
    pairs.append(("hi", 8, "lo", 8, 1, 8))
    return pairs


PAIRS = _pair_table()
NPAIRS = len(PAIRS)


def build_nc() -> bass.Bass:
    nc = bacc.Bacc()
    x_ext = nc.dram_tensor("x", [BPC, C_IN, 3 * PITCH], _DT_F8, kind="ExternalInput")
    w_ext = nc.dram_tensor("w", [C_IN, NPAIRS * 2 * C_OUT], _DT_F8, kind="ExternalInput")
    b_ext = nc.dram_tensor("b", [C_OUT, 1], _DT_F32, kind="ExternalInput")
    y_ext = nc.dram_tensor("y", [BPC, C_OUT, N], _DT_F32, kind="ExternalOutput")

    with tile.TileContext(nc) as tc:
        with (
            tc.tile_pool(name="const", bufs=1) as cpool,
            tc.tile_pool(name="xin", bufs=1) as xpool,
            tc.tile_pool(name="psum", bufs=8, space="PSUM") as ppool,
            # never-reused staging slots: ACT writes carry no WAR waits
            tc.tile_pool(name="out", bufs=OUT_PARTS * BPC - 1) as opool,
            tc.tile_pool(name="tail", bufs=1) as tpool,
        ):
            w_t = cpool.tile([C_IN, NPAIRS * 2, C_OUT], _DT_F8)
            bias_t = cpool.tile([C_OUT, 1], _DT_F32)
            x_tiles = []
            for b in range(BPC):
                xt = xpool.tile([C_IN, 3, PITCH], _DT_F8, tag=f"x{b}")
                x_tiles.append(xt)

            # (No p-state warmup needed: the cost model's PE clock ramp is
            # time-based here and the first real matmul issues past the 3us
            # full-speed threshold anyway.)

            # ---- head DMAs. Input x on the SP queue; w/bias (and later the
            # output stages) on the ACT queue so transfers run in parallel.
            # The matching dummy-LDWEIGHTS wait-absorbers are emitted at each
            # piece's first consumption point in the main loop, so the PE
            # never blocks on data it does not need yet.
            def x_piece(b, s, e):
                nc.sync.dma_start(
                    out=x_tiles[b][:, :, s:e],
                    in_=x_ext[b].rearrange("p (three f) -> p three f", three=3)[
                        :, :, s:e
                    ],
                )

            def x_ld(b, s):
                # dummy bf16 LDWEIGHTS inside piece (b, s..): absorbs the DMA
                # wait on the PE queue (cost-free; loaded weights never used)
                xbf = x_tiles[b][:].bitcast(mybir.dt.bfloat16)
                nc.tensor.ldweights(xbf[:, 0:1, s // 2 : s // 2 + C_OUT])

            # w pairs [0, W_SP) ride the SP queue AHEAD of the x pieces (the
            # SP path has ~0.6us less DGE latency); the rest + bias go via
            # ACT and land while chunk 0's chain is mid-flight.
            w3 = w_ext[:].rearrange("p (n f) -> p n f", n=NPAIRS * 2)
            w_cuts = (0, W_SP)
            nc.sync.dma_start(out=w_t[:, : 2 * W_SP, :], in_=w3[:, : 2 * W_SP, :])
            nc.scalar.dma_start(out=w_t[:, 2 * W_SP :, :], in_=w3[:, 2 * W_SP :, :])
            wbf = w_t[:].bitcast(mybir.dt.bfloat16)

            cuts0 = [0]
            for p in X0_PIECES:
                cuts0.append(cuts0[-1] + p)
            cuts0.append(PITCH)
            x0_cuts = []
            first_x0 = True
            for s, e in zip(cuts0[:-1], cuts0[1:]):
                if e > s:
                    x_piece(0, s, e)
                    x0_cuts.append(s)
                    if first_x0:
                        # bias rides SP right behind the head pieces (ACT's
                        # DGE path is slower and must deliver w pairs 6-12)
                        nc.sync.dma_start(out=bias_t[:], in_=b_ext[:])
                        first_x0 = False
            for b in range(1, BPC):
                for s, e in ((0, XH_SPLIT), (XH_SPLIT, PITCH)):
                    x_piece(b, s, e)

            # ---- main loop: per 512-col chunk, NPAIRS PSUM-accumulated DR
            # matmuls, then ACT identity+bias into an SBUF staging slot
            part = N // OUT_PARTS
            first_chunk = True
            for b in range(BPC):
                x_t = x_tiles[b]
                for h in range(OUT_PARTS):
                    last_part = b == BPC - 1 and h == OUT_PARTS - 1
                    head_part = b == 0 and h == 0
                    if last_part:
                        subs = list(TAIL_SUBS)
                    elif head_part:
                        subs = list(HEAD_SUBS)
                    else:
                        subs = [part]
                    off = 0
                    for ui, sub in enumerate(subs):
                        pool_ = tpool if last_part else opool
                        stage = pool_.tile(
                            [C_OUT, sub],
                            _DT_F32,
                            tag=f"tail{ui}" if last_part else "stage",
                        )
                        assert sub <= CHUNK or sub % CHUNK == 0
                        for cc in range(max(1, sub // CHUNK)):
                            w_cols = min(sub, CHUNK)
                            m0 = h * part + off + cc * w_cols
                            # wait-absorbers for data this chunk is first to use
                            if b == 0:
                                if m0 in X0_LD:
                                    x_ld(0, x0_cuts[X0_LD[m0]])
                            else:
                                if m0 == 0:
                                    x_ld(b, 0)
                                elif m0 == 2048:
                                    x_ld(b, XH_SPLIT)
                            ps = ppool.tile([C_OUT, w_cols], _DT_F32, tag="ps")
                            for pi, (_, _, _, _, plo, coff) in enumerate(PAIRS):
                                if first_chunk and pi in w_cuts:
                                    # each w piece lands mid-chain; absorb its
                                    # wait right before its first pair
                                    nc.tensor.ldweights(
                                        wbf[:, 2 * pi : 2 * pi + 2, 0 : C_OUT // 2]
                                    )
                                nc.tensor.matmul(
                                    ps[:],
                                    w_t[:, 2 * pi : 2 * pi + 2, :],
                                    x_t[:, plo : plo + 2, m0 + coff : m0 + coff + w_cols],
                                    start=(pi == 0),
                                    stop=(pi == NPAIRS - 1),
                                    perf_mode=_DR,
                                )
                            first_chunk = False
                            osl = stage[:, cc * w_cols : (cc + 1) * w_cols]
                            if last_part and ui >= 1:
                                # final pieces drain on the (idle) DVE so the
                                # tail starts the moment the PE stops
                                nc.vector.tensor_scalar_add(osl, ps[:], bias_t[:])
                            else:
                                nc.scalar.activation(
                                    osl,
                                    ps[:],
                                    mybir.ActivationFunctionType.Identity,
                                    bias=bias_t[:],
                                )
                        # tail-part outputs ride the (long idle) SP queue so
                        # their DGE config does not queue behind earlier
                        # output transfers on the ACT queue
                        dma_eng = nc.sync if last_part else nc.scalar
                        dma_eng.dma_start(
                            out=y_ext[b, :, h * part + off : h * part + off + sub],
                            in_=stage[:],
                        )
                        off += sub
    # Legalize: splits any instruction with >1 sync wait into EventSemaphore
    # chains (TRN2 allows one wait per instruction), register alloc, DCE.
    nc.compile()
    return nc


def _prep_inputs(x: np.ndarray, k: np.ndarray, bias: np.ndarray):
    x_hi8 = x.astype(_NP_F8)
    x_hi = x_hi8.astype(np.float32)
    x_lo8 = (x - x_hi).astype(_NP_F8)

    idx = np.arange(COLS_USED)
    planes = np.zeros((B, C_IN, 3, PITCH), dtype=_NP_F8)
    planes[:, :, 0, :COLS_USED] = x_hi8[:, :, (idx - 1) % N]
    planes[:, :, 1, :COLS_USED] = x_hi8[:, :, idx % N]
    planes[:, :, 2, :COLS_USED] = x_lo8[:, :, idx % N]
    planes = planes.reshape(B, C_IN, 3 * PITCH)

    k_hi8 = k.astype(_NP_F8)
    k_hi = k_hi8.astype(np.float32)
    k_lo8 = (k - k_hi).astype(_NP_F8)
    ksrc = {"hi": k_hi8, "lo": k_lo8}
    w = np.zeros((C_IN, NPAIRS * 2, C_OUT), dtype=_NP_F8)
    for pi, (ka, ta, kb, tb, _, _) in enumerate(PAIRS):
        w[:, 2 * pi, :] = ksrc[ka][:, :, ta].T  # [i, o]
        w[:, 2 * pi + 1, :] = ksrc[kb][:, :, tb].T
    w = w.reshape(C_IN, NPAIRS * 2 * C_OUT)

    b2 = np.ascontiguousarray(bias.reshape(C_OUT, 1)).astype(np.float32)
    return [
        {
            "x": np.ascontiguousarray(planes[c * BPC : (c + 1) * BPC]),
            "w": w,
            "b": b2,
        }
        for c in range(N_CORES)
    ]


_NC_CACHE = []


def kernel(**inputs: np.ndarray) -> np.ndarray:
    x = np.asarray(inputs["x"], dtype=np.float32)
    k = np.asarray(inputs["kernel"], dtype=np.float32)
    bias = np.asarray(inputs["bias"], dtype=np.float32)
    assert x.shape == (B, C_IN, N) and k.shape == (C_OUT, C_IN, KS)

    if not _NC_CACHE:
        _NC_CACHE.append(build_nc())
    nc = _NC_CACHE[0]

    in_maps = _prep_inputs(x, k, bias)
    res = run_bass_kernel_spmd(nc, in_maps, list(range(N_CORES)))
    y = np.concatenate([res.results[c]["y"] for c in range(N_CORES)], axis=0)
    return y.astype(np.float32, copy=False)
